# revision 4
# baseline (speedup 1.0000x reference)
"""Cross-attention kernel for 8 TRN2 NeuronCores.

Reference computation (per problem spec):
    q = (x @ Wq)  [B=4, N=4096, D=1024] -> heads [B, 16, N, 64]
    k = (context @ Wk), v = (context @ Wv)   context [B, M=256, 768]
    out = softmax(q k^T / 8 + mask) v   -> [B, N, D] @ Wo

Sharding: the 16384 query rows (B*N) are split evenly across the 8 cores
(2048 rows each, each shard living inside one batch). K/V are computed
redundantly per core from that core's batch context (only ~0.8 GFLOP) so no
collectives are needed; each core produces its own 2048 output rows and the
host concatenates them.

On-core dataflow (bf16 matmuls, fp32 PSUM):
  - x^T and context^T are pre-transposed on the HOST (free w.r.t. HW time),
    so the PE does zero transposes; all tensors arrive partition-major so
    every DMA is 128 large contiguous descriptors
  - DMAs are ordered by first-use: ctx^T + Wk first (K projection starts at
    ~2.5us), then x^T chunk 0 + Wq (Q proj), then Wv / Wo / later x chunks
  - kTe/kTo hold K with the other head's 64 partitions zeroed, so ONE
    full-width q tile serves both heads' score matmuls and Q eviction is a
    single [128,512] copy per group
  - steady state is a flat 32-step pipeline (4 q-chunks x 8 head pairs);
    each step interleaves on the PE: scores(c,i) | Wq proj(c+1, m=i) |
    AV(c,i-1) | out-proj group of chunk c-1 (lagged).  ScalarE exps overlap
    the projection matmuls instead of serializing the PE.
  - AV lhsT = [ones | V_h] so each AV matmul also emits the softmax
    denominator on partitions 0:63 (reciprocal_approx_fast needs base
    partition 0); VectorE does reciprocal + normalize
  - PSUM budget is exactly 8 banks: shared pool (scores/proj) x4 + av_e x2
    + av_o x2
"""

import sys

for _p in ("/opt/trn_rl_repo",):
    if _p not in sys.path:
        sys.path.insert(0, _p)

import numpy as np

import concourse.bass as bass
import concourse.mybir as mybir
import concourse.tile as tile
from concourse import bacc
from concourse.bass_utils import run_bass_kernel_spmd

ts = bass.ts

N_CORES = 8
B, N, D = 4, 4096, 1024
CTX = 768
M = 256          # kv length
H, HD = 16, 64   # heads, head dim
NQ = (B * N) // N_CORES   # 2048 query rows per core
QCH = 512                 # q chunk (free dim of most matmuls)
NQC = NQ // QCH           # 4 q chunks
NRB = NQ // 128           # 16 row-blocks
DT = D // 128             # 8 d-blocks (= head pairs)
KCH = CTX // 128          # 6 contraction chunks for context projections
F32 = mybir.dt.float32
BF16 = mybir.dt.bfloat16

SCALE = HD ** -0.5


def build_nc():
    nc = bacc.Bacc()

    xt_ext = nc.declare_dram_parameter("xt", [128, NQC, DT, QCH], BF16, isOutput=False)
    ctxt_ext = nc.declare_dram_parameter("ctxt", [128, KCH, M], BF16, isOutput=False)
    maskb_ext = nc.declare_dram_parameter("maskb", [128, 2], F32, isOutput=False)
    wq_ext = nc.declare_dram_parameter("wq", [128, DT, DT, 128], BF16, isOutput=False)
    wk_ext = nc.declare_dram_parameter("wk", [128, DT, KCH, 128], BF16, isOutput=False)
    wv_ext = nc.declare_dram_parameter("wv", [128, 2, KCH, QCH], BF16, isOutput=False)
    wo_ext = nc.declare_dram_parameter("wo", [128, 2, DT, QCH], BF16, isOutput=False)
    out_ext = nc.declare_dram_parameter("out", [128, NRB, D], BF16, isOutput=True)

    with tile.TileContext(nc) as tc:
        # ---- persistent tensors -------------------------------------------
        mask_sb, free_mask = tc.tile([128, 2], F32, name="mask_sb")
        ctxt_sb, free_ctxt = tc.tile([128, KCH, M], BF16, name="ctxt_sb")
        xt_sb, free_xt = tc.tile([128, NQC, DT, QCH], BF16, name="xt_sb")
        # K per head pair, with the other head's partitions zeroed: kTe has
        # head 2i dims on partitions 0:63 (64:128 zero), kTo has head 2i+1 on
        # 64:128 (0:64 zero).  A score matmul with full-width q then yields an
        # exact single-head result with full 128-partition contraction.
        kTe, free_kTe = tc.tile([128, DT, M], BF16, name="kTe")
        kTo, free_kTo = tc.tile([128, DT, M], BF16, name="kTo")
        # per (kv-block j, head h): [ones | V_h]; the ones columns make each
        # AV matmul also emit the softmax denominator on partitions 0:63
        vvx, free_vvx = tc.tile([128, 2, H, 128], BF16, name="vvx")
        qT, free_qT = tc.tile([128, DT, 2, QCH], BF16, name="qT")
        oT, free_oT = tc.tile([128, DT, 3, QCH], BF16, name="oT")
        wq_sb, free_wq = tc.tile([128, DT, DT, 128], BF16, name="wq_sb")
        wk_sb, free_wk = tc.tile([128, DT, KCH, 128], BF16, name="wk_sb")
        wv_sb, free_wv = tc.tile([128, 2, KCH, QCH], BF16, name="wv_sb")
        wo_sb, free_wo = tc.tile([128, 2, DT, QCH], BF16, name="wo_sb")

        # one-time zero/ones fills on the (otherwise idle) gpsimd engine
        nc.gpsimd.memset(kTe[64:128, :, :], 0.0)
        nc.gpsimd.memset(kTo[0:64, :, :], 0.0)
        nc.gpsimd.memset(vvx[:, :, :, 0:HD], 1.0)

        with tc.tile_pool(name="attnp", bufs=4) as attnp, \
             tc.tile_pool(name="recp", bufs=2) as recp, \
             tc.tile_pool(name="outp", bufs=2) as outp, \
             tc.tile_pool(name="mpsum", bufs=4, space="PSUM") as mpsum:
            # ---- input DMAs, ordered by first use.
            # sync ring: ctx^T, mask, x^T chunks.  scalar ring: Wk, Wq
            # (per-m so the projections can start as soon as the first
            # 128-column group lands), then Wv, Wo.
            with tc.high_priority():
                nc.sync.dma_start(out=ctxt_sb, in_=ctxt_ext[:, :, :])
                nc.gpsimd.dma_start(out=mask_sb, in_=maskb_ext[:, :])
                nc.sync.dma_start(out=xt_sb[:, 0], in_=xt_ext[:, 0])
                for m in range(DT):
                    nc.scalar.dma_start(out=wk_sb[:, m], in_=wk_ext[:, m])
                for m in range(DT):
                    nc.scalar.dma_start(out=wq_sb[:, m], in_=wq_ext[:, m])
            for n in range(2):
                nc.sync.dma_start(out=wv_sb[:, n], in_=wv_ext[:, n])
            for c in range(1, NQC):
                nc.sync.dma_start(out=xt_sb[:, c], in_=xt_ext[:, c])
            for n in range(2):
                nc.scalar.dma_start(out=wo_sb[:, n], in_=wo_ext[:, n])

            # ---- prologue: K projection (earliest data), then Q chunk 0,
            # then V (wv arrives after wq) ----------------------------------
            for m in range(DT):
                psk = mpsum.tile([128, QCH], F32, name="psk", tag="ps")
                for k in range(KCH):
                    nc.tensor.matmul(
                        psk[:, 0:M], wk_sb[:, m, k, :], ctxt_sb[:, k, :],
                        start=(k == 0), stop=(k == KCH - 1),
                    )
                nc.vector.tensor_copy(kTe[0:64, m, :], psk[0:64, 0:M])
                nc.vector.tensor_copy(kTo[64:128, m, :], psk[64:128, 0:M])

            def do_qproj(c, m):
                ps = mpsum.tile([128, QCH], F32, name="ps_q", tag="ps")
                for k in range(DT):
                    nc.tensor.matmul(
                        ps[:, :], wq_sb[:, m, k, :], xt_sb[:, c, k, :],
                        start=(k == 0), stop=(k == DT - 1),
                    )
                nc.scalar.activation(
                    qT[:, m, c % 2, :], ps,
                    mybir.ActivationFunctionType.Copy,
                )

            for m in range(DT):
                do_qproj(0, m)

            def do_v(j, n):
                psv = mpsum.tile([128, 8, HD], F32, name="psv", tag="ps")
                for k in range(KCH):
                    nc.tensor.matmul(
                        psv[:, :, :], ctxt_sb[:, k, ts(j, 128)],
                        wv_sb[:, n, k, :],
                        start=(k == 0), stop=(k == KCH - 1),
                    )
                nc.vector.tensor_copy(vvx[:, j, 8 * n : 8 * n + 8, HD:128], psv)

            # ---- steady pipeline ------------------------------------------
            # state for lagged av/out-proj emission
            def emit_sc_exp(c, i, j, odd):
                kt = kTo if odd else kTe
                sc = mpsum.tile([128, QCH], F32, name="sc", tag="ps")
                nc.tensor.matmul(
                    sc[:, :], kt[:, i, ts(j, 128)], qT[:, i, c % 2, :],
                    start=True, stop=True,
                )
                at = attnp.tile(
                    [128, QCH], BF16, name="at",
                    tag=("at_o" if odd else "at_e"),
                )
                nc.scalar.activation(
                    at, sc, mybir.ActivationFunctionType.Exp,
                    bias=mask_sb[:, j : j + 1], scale=SCALE,
                )
                return at

            def emit_av(c, i, ats):
                # ats: {(j, odd): at tile}
                av_e = mpsum.tile([128, QCH], F32, name="av_e", tag="av_e", bufs=2)
                av_o = mpsum.tile([128, QCH], F32, name="av_o", tag="av_o", bufs=2)
                for j in range(2):
                    nc.tensor.matmul(
                        av_e[:, :], vvx[:, j, 2 * i, :], ats[(j, 0)],
                        start=(j == 0), stop=(j == 1),
                    )
                for j in range(2):
                    nc.tensor.matmul(
                        av_o[:, :], vvx[:, j, 2 * i + 1, :], ats[(j, 1)],
                        start=(j == 0), stop=(j == 1),
                    )
                rec_e = recp.tile([64, QCH], F32, name="rec_e", tag="rec_e")
                rec_o = recp.tile([64, QCH], F32, name="rec_o", tag="rec_o")
                nc.vector.reciprocal_approx_fast(rec_e, av_e[0:64, :])
                nc.vector.reciprocal_approx_fast(rec_o, av_o[0:64, :])
                cc3 = c % 3
                nc.vector.tensor_mul(oT[0:64, i, cc3, :], av_e[64:128, :], rec_e)
                nc.vector.tensor_mul(oT[64:128, i, cc3, :], av_o[64:128, :], rec_o)

            ob_cur = [None]

            def emit_outproj(c, g):
                # out rows (chunk c, row-block g>>1, half g&1) = oT.T @ Wo
                mr, n = g >> 1, g & 1
                split = c == NQC - 1  # store halves eagerly to shrink the tail
                if n == 0 and not split:
                    ob_cur[0] = outp.tile([128, D], BF16, name="ob", tag="ob")
                ob = ob_cur[0]
                ops = mpsum.tile([128, QCH], F32, name="ops", tag="ps")
                for k in range(DT):
                    nc.tensor.matmul(
                        ops[:, :], oT[:, k, c % 3, ts(mr, 128)],
                        wo_sb[:, n, k, :],
                        start=(k == 0), stop=(k == DT - 1),
                    )
                if split:
                    obh = outp.tile([128, QCH], BF16, name="obh", tag="obh")
                    if n == 0:
                        nc.vector.tensor_copy(obh, ops)
                    else:
                        nc.scalar.activation(
                            obh, ops, mybir.ActivationFunctionType.Copy
                        )
                    nc.sync.dma_start(
                        out=out_ext[:, 4 * c + mr, ts(n, QCH)], in_=obh
                    )
                elif n == 0:
                    nc.vector.tensor_copy(ob[:, ts(n, QCH)], ops)
                else:
                    nc.scalar.activation(
                        ob[:, ts(n, QCH)], ops,
                        mybir.ActivationFunctionType.Copy,
                    )
                    nc.sync.dma_start(out=out_ext[:, 4 * c + mr, :], in_=ob)

            prev_ats = None
            for c in range(NQC):
                for i in range(DT):
                    # scores j=0 for (c, i)
                    ats = {}
                    ats[(0, 0)] = emit_sc_exp(c, i, 0, 0)
                    ats[(0, 1)] = emit_sc_exp(c, i, 0, 1)
                    # first half of next-chunk Q projection group m=i
                    if c + 1 < NQC:
                        ps_q = mpsum.tile([128, QCH], F32, name="ps_q", tag="ps")
                        for k in range(4):
                            nc.tensor.matmul(
                                ps_q[:, :], wq_sb[:, i, k, :],
                                xt_sb[:, c + 1, k, :],
                                start=(k == 0), stop=False,
                            )
                    # scores j=1
                    ats[(1, 0)] = emit_sc_exp(c, i, 1, 0)
                    ats[(1, 1)] = emit_sc_exp(c, i, 1, 1)
                    # second half of Q projection + eviction
                    if c + 1 < NQC:
                        for k in range(4, DT):
                            nc.tensor.matmul(
                                ps_q[:, :], wq_sb[:, i, k, :],
                                xt_sb[:, c + 1, k, :],
                                start=False, stop=(k == DT - 1),
                            )
                        nc.scalar.activation(
                            qT[:, i, (c + 1) % 2, :], ps_q,
                            mybir.ActivationFunctionType.Copy,
                        )
                    # AV + normalize for the previous pair
                    if i > 0:
                        emit_av(c, i - 1, prev_ats)
                    elif c > 0:
                        emit_av(c - 1, DT - 1, prev_ats)
                    prev_ats = ats
                    # lagged out-projection of chunk c-1 (groups 0..5 at
                    # steps 2..7, groups 6..7 at the next chunk's steps 0..1)
                    if i < 2:
                        if c >= 2:
                            emit_outproj(c - 2, 6 + i)
                    else:
                        if c >= 1:
                            emit_outproj(c - 1, i - 2)
                    # V projection rides between the first two steps of
                    # chunk 0 (wv lands after wq)
                    if c == 0 and i == 0:
                        do_v(0, 0)
                        do_v(1, 0)
                    elif c == 0 and i == 1:
                        do_v(0, 1)
                        do_v(1, 1)

            # ---- epilogue --------------------------------------------------
            emit_av(NQC - 1, DT - 1, prev_ats)
            emit_outproj(NQC - 2, 6)
            emit_outproj(NQC - 2, 7)
            for g in range(8):
                emit_outproj(NQC - 1, g)

        # release singles in reverse allocation order
        free_wo()
        free_wv()
        free_wk()
        free_wq()
        free_oT()
        free_qT()
        free_vvx()
        free_kTo()
        free_kTe()
        free_xt()
        free_ctxt()
        free_mask()

    nc.finalize()
    return nc


_NC_CACHE = None


def _get_nc():
    global _NC_CACHE
    if _NC_CACHE is None:
        _NC_CACHE = build_nc()
    return _NC_CACHE


def make_in_maps(x, context, context_mask, Wq, Wk, Wv, Wo):
    import ml_dtypes

    bf = ml_dtypes.bfloat16
    x = np.asarray(x, dtype=np.float32)
    context = np.asarray(context, dtype=np.float32)
    mask = np.asarray(context_mask)

    # additive exp-bias per kv position: 0 where visible, -1e9 where masked
    bias = (mask.astype(np.float32) - 1.0) * 1e9          # [B, M]
    x_flat = x.reshape(B * N, D)

    # weights, partition-major with per-output-group contiguity
    wq_s = np.ascontiguousarray(
        np.asarray(Wq, np.float32).reshape(DT, 128, DT, 128).transpose(1, 2, 0, 3)
    ).astype(bf)
    wk_s = np.ascontiguousarray(
        np.asarray(Wk, np.float32).reshape(KCH, 128, DT, 128).transpose(1, 2, 0, 3)
    ).astype(bf)
    wv_s = np.ascontiguousarray(
        np.asarray(Wv, np.float32).reshape(KCH, 128, 2, QCH).transpose(1, 2, 0, 3)
    ).astype(bf)
    wo_s = np.ascontiguousarray(
        np.asarray(Wo, np.float32).reshape(DT, 128, 2, QCH).transpose(1, 2, 0, 3)
    ).astype(bf)

    in_maps = []
    for c in range(N_CORES):
        b = (c * NQ) // N
        shard = x_flat[c * NQ : (c + 1) * NQ]
        # xt[p, cc, k, n] = shard[512*cc + n, 128*k + p]
        xt = np.ascontiguousarray(
            shard.reshape(NQC, QCH, DT, 128).transpose(3, 0, 2, 1)
        ).astype(bf)
        # ctxt[p, k, j] = context[b, j, 128*k + p]
        ctxt = np.ascontiguousarray(
            context[b].reshape(M, KCH, 128).transpose(2, 1, 0)
        ).astype(bf)
        in_maps.append({
            "xt": xt,
            "ctxt": ctxt,
            "maskb": np.ascontiguousarray(bias[b].reshape(2, 128).T),
            "wq": wq_s, "wk": wk_s, "wv": wv_s, "wo": wo_s,
        })
    return in_maps


def kernel(x, context, context_mask, Wq, Wk, Wv, Wo):
    nc = _get_nc()
    in_maps = make_in_maps(x, context, context_mask, Wq, Wk, Wv, Wo)
    res = run_bass_kernel_spmd(nc, in_maps, core_ids=list(range(N_CORES)))
    # out arrives partition-major bf16: [128, NRB, D] per core
    outs = []
    for c in range(N_CORES):
        o = np.asarray(res.results[c]["out"], dtype=np.float32)
        outs.append(o.transpose(1, 0, 2).reshape(NQ, D))
    return np.concatenate(outs, axis=0).reshape(B, N, D)


# revision 5
# speedup vs baseline: 1.0190x; 1.0190x over previous
"""Cross-attention kernel for 8 TRN2 NeuronCores.

Reference computation (per problem spec):
    q = (x @ Wq)  [B=4, N=4096, D=1024] -> heads [B, 16, N, 64]
    k = (context @ Wk), v = (context @ Wv)   context [B, M=256, 768]
    out = softmax(q k^T / 8 + mask) v   -> [B, N, D] @ Wo

Sharding: the 16384 query rows (B*N) are split evenly across the 8 cores
(2048 rows each, each shard living inside one batch). K/V are computed
redundantly per core from that core's batch context (only ~0.8 GFLOP) so no
collectives are needed; each core produces its own 2048 output rows and the
host concatenates them.

On-core dataflow (bf16 matmuls, fp32 PSUM):
  - x^T and context^T are pre-transposed on the HOST (free w.r.t. HW time),
    so the PE does zero transposes; all tensors arrive partition-major so
    every DMA is 128 large contiguous descriptors
  - DMAs are ordered by first-use: ctx^T + Wk first (K projection starts at
    ~2.5us), then x^T chunk 0 + Wq (Q proj), then Wv / Wo / later x chunks
  - kTe/kTo hold K with the other head's 64 partitions zeroed, so ONE
    full-width q tile serves both heads' score matmuls and Q eviction is a
    single [128,512] copy per group
  - steady state is a flat 32-step pipeline (4 q-chunks x 8 head pairs);
    each step interleaves on the PE: scores(c,i) | Wq proj(c+1, m=i) |
    AV(c,i-1) | out-proj group of chunk c-1 (lagged).  ScalarE exps overlap
    the projection matmuls instead of serializing the PE.
  - AV lhsT = [ones | V_h] so each AV matmul also emits the softmax
    denominator on partitions 0:63 (reciprocal_approx_fast needs base
    partition 0); VectorE does reciprocal + normalize
  - PSUM budget is exactly 8 banks: shared pool (scores/proj) x4 + av_e x2
    + av_o x2
"""

import sys

for _p in ("/opt/trn_rl_repo",):
    if _p not in sys.path:
        sys.path.insert(0, _p)

import numpy as np

import concourse.bass as bass
import concourse.mybir as mybir
import concourse.tile as tile
from concourse import bacc
from concourse.bass_utils import run_bass_kernel_spmd

ts = bass.ts

N_CORES = 8
B, N, D = 4, 4096, 1024
CTX = 768
M = 256          # kv length
H, HD = 16, 64   # heads, head dim
NQ = (B * N) // N_CORES   # 2048 query rows per core
QCH = 512                 # q chunk (free dim of most matmuls)
NQC = NQ // QCH           # 4 q chunks
NRB = NQ // 128           # 16 row-blocks
DT = D // 128             # 8 d-blocks (= head pairs)
KCH = CTX // 128          # 6 contraction chunks for context projections
F32 = mybir.dt.float32
BF16 = mybir.dt.bfloat16

SCALE = HD ** -0.5


def build_nc():
    nc = bacc.Bacc()

    xt_ext = nc.declare_dram_parameter("xt", [128, NQC, DT, QCH], BF16, isOutput=False)
    ctxt_ext = nc.declare_dram_parameter("ctxt", [128, KCH, M], BF16, isOutput=False)
    maskb_ext = nc.declare_dram_parameter("maskb", [128, 2], F32, isOutput=False)
    wq_ext = nc.declare_dram_parameter("wq", [128, DT, DT, 128], BF16, isOutput=False)
    wk_ext = nc.declare_dram_parameter("wk", [128, DT, KCH, 128], BF16, isOutput=False)
    wv_ext = nc.declare_dram_parameter("wv", [128, 2, KCH, QCH], BF16, isOutput=False)
    wo_ext = nc.declare_dram_parameter("wo", [128, 2, DT, QCH], BF16, isOutput=False)
    out_ext = nc.declare_dram_parameter("out", [128, NRB, D], BF16, isOutput=True)

    with tile.TileContext(nc) as tc:
        # ---- persistent tensors -------------------------------------------
        mask_sb, free_mask = tc.tile([128, 2], F32, name="mask_sb")
        ctxt_sb, free_ctxt = tc.tile([128, KCH, M], BF16, name="ctxt_sb")
        xt_sb, free_xt = tc.tile([128, NQC, DT, QCH], BF16, name="xt_sb")
        # K per head pair, with the other head's partitions zeroed: kTe has
        # head 2i dims on partitions 0:63 (64:128 zero), kTo has head 2i+1 on
        # 64:128 (0:64 zero).  A score matmul with full-width q then yields an
        # exact single-head result with full 128-partition contraction.
        kTe, free_kTe = tc.tile([128, DT, M], BF16, name="kTe")
        kTo, free_kTo = tc.tile([128, DT, M], BF16, name="kTo")
        # per (kv-block j, head h): [ones | V_h]; the ones columns make each
        # AV matmul also emit the softmax denominator on partitions 0:63
        vvx, free_vvx = tc.tile([128, 2, H, 128], BF16, name="vvx")
        qT, free_qT = tc.tile([128, DT, 2, QCH], BF16, name="qT")
        oT, free_oT = tc.tile([128, DT, 3, QCH], BF16, name="oT")
        wq_sb, free_wq = tc.tile([128, DT, DT, 128], BF16, name="wq_sb")
        wk_sb, free_wk = tc.tile([128, DT, KCH, 128], BF16, name="wk_sb")
        wv_sb, free_wv = tc.tile([128, 2, KCH, QCH], BF16, name="wv_sb")
        wo_sb, free_wo = tc.tile([128, 2, DT, QCH], BF16, name="wo_sb")

        # one-time zero/ones fills on the (otherwise idle) gpsimd engine
        nc.gpsimd.memset(kTe[64:128, :, :], 0.0)
        nc.gpsimd.memset(kTo[0:64, :, :], 0.0)
        nc.gpsimd.memset(vvx[:, :, :, 0:HD], 1.0)

        with tc.tile_pool(name="attnp", bufs=4) as attnp, \
             tc.tile_pool(name="recp", bufs=2) as recp, \
             tc.tile_pool(name="outp", bufs=2) as outp, \
             tc.tile_pool(name="mpsum", bufs=4, space="PSUM") as mpsum:
            # ---- input DMAs, ordered by first use.
            # sync ring: ctx^T, mask, x^T chunks.  scalar ring: Wk, Wq
            # (per-m so the projections can start as soon as the first
            # 128-column group lands), then Wv, Wo.
            # ring balance: scalar = ctxt + Wk + Wq + Wo (5.9 MB, need-ordered),
            # sync = x^T chunks + Wv (5.9 MB).  mask rides the idle gpsimd
            # SWDGE queue.
            with tc.high_priority():
                nc.scalar.dma_start(out=ctxt_sb[:, 0:3, :], in_=ctxt_ext[:, 0:3, :])
                nc.scalar.dma_start(out=ctxt_sb[:, 3:6, :], in_=ctxt_ext[:, 3:6, :])
                nc.gpsimd.dma_start(out=mask_sb, in_=maskb_ext[:, :])
                nc.sync.dma_start(out=xt_sb[:, 0], in_=xt_ext[:, 0])
                for m in range(DT):
                    nc.scalar.dma_start(out=wk_sb[:, m], in_=wk_ext[:, m])
                for m in range(DT):
                    nc.scalar.dma_start(out=wq_sb[:, m], in_=wq_ext[:, m])
            nc.sync.dma_start(out=xt_sb[:, 1], in_=xt_ext[:, 1])
            for n in range(2):
                nc.sync.dma_start(out=wv_sb[:, n], in_=wv_ext[:, n])
            for c in range(2, NQC):
                nc.sync.dma_start(out=xt_sb[:, c], in_=xt_ext[:, c])
            for n in range(2):
                nc.scalar.dma_start(out=wo_sb[:, n], in_=wo_ext[:, n])

            # ---- prologue: K projection (earliest data), then Q chunk 0,
            # then V (wv arrives after wq) ----------------------------------
            for m in range(DT):
                psk = mpsum.tile([128, QCH], F32, name="psk", tag="ps")
                for k in range(KCH):
                    nc.tensor.matmul(
                        psk[:, 0:M], wk_sb[:, m, k, :], ctxt_sb[:, k, :],
                        start=(k == 0), stop=(k == KCH - 1),
                    )
                nc.vector.tensor_copy(kTe[0:64, m, :], psk[0:64, 0:M])
                nc.vector.tensor_copy(kTo[64:128, m, :], psk[64:128, 0:M])

            def do_qproj(c, m):
                ps = mpsum.tile([128, QCH], F32, name="ps_q", tag="ps")
                for k in range(DT):
                    nc.tensor.matmul(
                        ps[:, :], wq_sb[:, m, k, :], xt_sb[:, c, k, :],
                        start=(k == 0), stop=(k == DT - 1),
                    )
                nc.scalar.activation(
                    qT[:, m, c % 2, :], ps,
                    mybir.ActivationFunctionType.Copy,
                )

            for m in range(DT):
                do_qproj(0, m)

            def do_v(j, n):
                psv = mpsum.tile([128, 8, HD], F32, name="psv", tag="ps")
                for k in range(KCH):
                    nc.tensor.matmul(
                        psv[:, :, :], ctxt_sb[:, k, ts(j, 128)],
                        wv_sb[:, n, k, :],
                        start=(k == 0), stop=(k == KCH - 1),
                    )
                nc.vector.tensor_copy(vvx[:, j, 8 * n : 8 * n + 8, HD:128], psv)

            # ---- steady pipeline ------------------------------------------
            # state for lagged av/out-proj emission
            def emit_sc_exp(c, i, j, odd):
                kt = kTo if odd else kTe
                sc = mpsum.tile([128, QCH], F32, name="sc", tag="ps")
                nc.tensor.matmul(
                    sc[:, :], kt[:, i, ts(j, 128)], qT[:, i, c % 2, :],
                    start=True, stop=True,
                )
                at = attnp.tile(
                    [128, QCH], BF16, name="at",
                    tag=("at_o" if odd else "at_e"),
                )
                nc.scalar.activation(
                    at, sc, mybir.ActivationFunctionType.Exp,
                    bias=mask_sb[:, j : j + 1], scale=SCALE,
                )
                return at

            def emit_av(c, i, ats):
                # ats: {(j, odd): at tile}
                av_e = mpsum.tile([128, QCH], F32, name="av_e", tag="av_e", bufs=2)
                av_o = mpsum.tile([128, QCH], F32, name="av_o", tag="av_o", bufs=2)
                for j in range(2):
                    nc.tensor.matmul(
                        av_e[:, :], vvx[:, j, 2 * i, :], ats[(j, 0)],
                        start=(j == 0), stop=(j == 1),
                    )
                for j in range(2):
                    nc.tensor.matmul(
                        av_o[:, :], vvx[:, j, 2 * i + 1, :], ats[(j, 1)],
                        start=(j == 0), stop=(j == 1),
                    )
                rec_e = recp.tile([64, QCH], F32, name="rec_e", tag="rec_e")
                rec_o = recp.tile([64, QCH], F32, name="rec_o", tag="rec_o")
                nc.vector.reciprocal_approx_fast(rec_e, av_e[0:64, :])
                nc.vector.reciprocal_approx_fast(rec_o, av_o[0:64, :])
                cc3 = c % 3
                nc.vector.tensor_mul(oT[0:64, i, cc3, :], av_e[64:128, :], rec_e)
                nc.vector.tensor_mul(oT[64:128, i, cc3, :], av_o[64:128, :], rec_o)

            ob_cur = [None]

            def emit_outproj(c, g):
                # out rows (chunk c, row-block g>>1, half g&1) = oT.T @ Wo
                mr, n = g >> 1, g & 1
                split = c == NQC - 1  # store halves eagerly to shrink the tail
                if n == 0 and not split:
                    ob_cur[0] = outp.tile([128, D], BF16, name="ob", tag="ob")
                ob = ob_cur[0]
                ops = mpsum.tile([128, QCH], F32, name="ops", tag="ps")
                for k in range(DT):
                    nc.tensor.matmul(
                        ops[:, :], oT[:, k, c % 3, ts(mr, 128)],
                        wo_sb[:, n, k, :],
                        start=(k == 0), stop=(k == DT - 1),
                    )
                if split:
                    obh = outp.tile([128, QCH], BF16, name="obh", tag="obh")
                    if n == 0:
                        nc.vector.tensor_copy(obh, ops)
                    else:
                        nc.scalar.activation(
                            obh, ops, mybir.ActivationFunctionType.Copy
                        )
                    nc.sync.dma_start(
                        out=out_ext[:, 4 * c + mr, ts(n, QCH)], in_=obh
                    )
                elif n == 0:
                    nc.vector.tensor_copy(ob[:, ts(n, QCH)], ops)
                else:
                    nc.scalar.activation(
                        ob[:, ts(n, QCH)], ops,
                        mybir.ActivationFunctionType.Copy,
                    )
                    nc.sync.dma_start(out=out_ext[:, 4 * c + mr, :], in_=ob)

            prev_ats = None
            for c in range(NQC):
                for i in range(DT):
                    # scores j=0 for (c, i)
                    ats = {}
                    ats[(0, 0)] = emit_sc_exp(c, i, 0, 0)
                    ats[(0, 1)] = emit_sc_exp(c, i, 0, 1)
                    # first half of next-chunk Q projection group m=i
                    if c + 1 < NQC:
                        ps_q = mpsum.tile([128, QCH], F32, name="ps_q", tag="ps")
                        for k in range(4):
                            nc.tensor.matmul(
                                ps_q[:, :], wq_sb[:, i, k, :],
                                xt_sb[:, c + 1, k, :],
                                start=(k == 0), stop=False,
                            )
                    # scores j=1
                    ats[(1, 0)] = emit_sc_exp(c, i, 1, 0)
                    ats[(1, 1)] = emit_sc_exp(c, i, 1, 1)
                    # second half of Q projection + eviction
                    if c + 1 < NQC:
                        for k in range(4, DT):
                            nc.tensor.matmul(
                                ps_q[:, :], wq_sb[:, i, k, :],
                                xt_sb[:, c + 1, k, :],
                                start=False, stop=(k == DT - 1),
                            )
                        nc.scalar.activation(
                            qT[:, i, (c + 1) % 2, :], ps_q,
                            mybir.ActivationFunctionType.Copy,
                        )
                    # AV + normalize for the previous pair
                    if i > 0:
                        emit_av(c, i - 1, prev_ats)
                    elif c > 0:
                        emit_av(c - 1, DT - 1, prev_ats)
                    prev_ats = ats
                    # lagged out-projection of chunk c-1 (groups 0..5 at
                    # steps 2..7, groups 6..7 at the next chunk's steps 0..1)
                    if i < 2:
                        if c >= 2:
                            emit_outproj(c - 2, 6 + i)
                    else:
                        if c >= 1:
                            emit_outproj(c - 1, i - 2)
                    # V projection rides between the first two steps of
                    # chunk 0 (wv lands after wq)
                    if c == 0 and i == 0:
                        do_v(0, 0)
                        do_v(1, 0)
                    elif c == 0 and i == 1:
                        do_v(0, 1)
                        do_v(1, 1)

            # ---- epilogue --------------------------------------------------
            emit_av(NQC - 1, DT - 1, prev_ats)
            emit_outproj(NQC - 2, 6)
            emit_outproj(NQC - 2, 7)
            for g in range(8):
                emit_outproj(NQC - 1, g)

        # release singles in reverse allocation order
        free_wo()
        free_wv()
        free_wk()
        free_wq()
        free_oT()
        free_qT()
        free_vvx()
        free_kTo()
        free_kTe()
        free_xt()
        free_ctxt()
        free_mask()

    nc.finalize()
    return nc


_NC_CACHE = None


def _get_nc():
    global _NC_CACHE
    if _NC_CACHE is None:
        _NC_CACHE = build_nc()
    return _NC_CACHE


def make_in_maps(x, context, context_mask, Wq, Wk, Wv, Wo):
    import ml_dtypes

    bf = ml_dtypes.bfloat16
    x = np.asarray(x, dtype=np.float32)
    context = np.asarray(context, dtype=np.float32)
    mask = np.asarray(context_mask)

    # additive exp-bias per kv position: 0 where visible, -1e9 where masked
    bias = (mask.astype(np.float32) - 1.0) * 1e9          # [B, M]
    x_flat = x.reshape(B * N, D)

    # weights, partition-major with per-output-group contiguity
    wq_s = np.ascontiguousarray(
        np.asarray(Wq, np.float32).reshape(DT, 128, DT, 128).transpose(1, 2, 0, 3)
    ).astype(bf)
    wk_s = np.ascontiguousarray(
        np.asarray(Wk, np.float32).reshape(KCH, 128, DT, 128).transpose(1, 2, 0, 3)
    ).astype(bf)
    wv_s = np.ascontiguousarray(
        np.asarray(Wv, np.float32).reshape(KCH, 128, 2, QCH).transpose(1, 2, 0, 3)
    ).astype(bf)
    wo_s = np.ascontiguousarray(
        np.asarray(Wo, np.float32).reshape(DT, 128, 2, QCH).transpose(1, 2, 0, 3)
    ).astype(bf)

    in_maps = []
    for c in range(N_CORES):
        b = (c * NQ) // N
        shard = x_flat[c * NQ : (c + 1) * NQ]
        # xt[p, cc, k, n] = shard[512*cc + n, 128*k + p]
        xt = np.ascontiguousarray(
            shard.reshape(NQC, QCH, DT, 128).transpose(3, 0, 2, 1)
        ).astype(bf)
        # ctxt[p, k, j] = context[b, j, 128*k + p]
        ctxt = np.ascontiguousarray(
            context[b].reshape(M, KCH, 128).transpose(2, 1, 0)
        ).astype(bf)
        in_maps.append({
            "xt": xt,
            "ctxt": ctxt,
            "maskb": np.ascontiguousarray(bias[b].reshape(2, 128).T),
            "wq": wq_s, "wk": wk_s, "wv": wv_s, "wo": wo_s,
        })
    return in_maps


def kernel(x, context, context_mask, Wq, Wk, Wv, Wo):
    nc = _get_nc()
    in_maps = make_in_maps(x, context, context_mask, Wq, Wk, Wv, Wo)
    res = run_bass_kernel_spmd(nc, in_maps, core_ids=list(range(N_CORES)))
    # out arrives partition-major bf16: [128, NRB, D] per core
    outs = []
    for c in range(N_CORES):
        o = np.asarray(res.results[c]["out"], dtype=np.float32)
        outs.append(o.transpose(1, 0, 2).reshape(NQ, D))
    return np.concatenate(outs, axis=0).reshape(B, N, D)
